# revision 12
# baseline (speedup 1.0000x reference)
"""Trainium2 Bass kernel for nn_Block_local (local windowed attention block).

Per-batch computation (reference semantics):
    q = LN(query + query_embed) -> 1x1 conv wq     (LN over channels, shared g/b)
    k = LN(key + key_embed)     -> 1x1 conv wk
    v = wv @ key + bv                               (conv on the RAW key)
    w[n, j] = sum_c q[c,n] * k_pad[c, n+j-pad]      j in [0, kH)
    w = softmax_j(w) * C**-0.5
    attn[c,n] = sum_j w[n,j] * v_pad[c, n+j-pad]
    x = query + attn
    x = x + MLP(LN2(x))                             (MLP: gelu(x@w1+b1)@w2+b2)

Sharding: data-parallel over batch B=8 across the 8 NeuronCores (one batch
per core); every core runs an identical program on its own batch slice.

Device-side algebra (host pre-folds all affine pieces):
  - LN gain/bias + q/k conv weights/biases fold into one similarity matrix
    in augmented space z = [xhat; 1]:  G = z_q^T (Aq^T Ak) z_k with
    Aq = [wq*g | wq@b_norm + bq].  Device computes kz = Mz @ z_k (lhsT=MzT),
    then banded G blocks via z_q^T @ kz windows.
  - Window = banded gram per 128-row block: affine_select band mask, band
    softmax (exp w/ accum_out), PE transpose of the weights, banded weighting
    matmul vs vT with an on-the-fly DMA halo for the upper band piece.
  - v bias bv rides the residual: softmax rows sum to `scale`, so the host
    adds scale*bv into query^T (the qTb input).
  - MLP phase re-loads the residual from a DRAM bounce (x2d), does LN2,
    PE-transposes to C layout, mm1 + Gelu(bias=c1), and mm2 emitting
    T-layout output directly with b2 via an augmented ones-row matmul and
    the final residual add fused behind it.

All big matmuls use float32r (PE fast path: 1 cycle/row vs 4 for fp32;
~1.4e-4 relative error, verified on HW). HW Gelu == erf gelu (verified).
"""

from contextlib import ExitStack

import numpy as np

import concourse.bass as bass
import concourse.tile as tile
from concourse import bacc, mybir
from concourse.bass_utils import run_bass_kernel_spmd
from concourse.masks import make_identity

f32 = mybir.dt.float32
f32r = mybir.dt.float32r
AF = mybir.ActivationFunctionType
ALU = mybir.AluOpType
AX = mybir.AxisListType

P = 128          # partitions
C = 512          # channels
H = 4 * C        # mlp hidden
EPS = 1e-5
NEG = -1e30

CT = C // P      # channel tiles (4)
HT = H // P      # mlp hidden tiles (16)

ts = bass.ts


def build_block_kernel(nc, N, KH, gelu_func=AF.Gelu):
    """Emit the tile program. N = sequence length, KH = window size (odd)."""
    NT = N // P               # n tiles
    PADW = KH // 2            # 4
    W = P + KH - 1            # band tile width (136)
    NCH = N // 512            # 512-wide column chunks
    scale = C ** -0.5

    # ---- DRAM I/O ----
    dI = {}
    for nm, shp, dt in [
        ("qT", [N, C], f32), ("qeT", [N, C], f32), ("kT", [N, C], f32),
        ("keT", [N, C], f32), ("qTb", [N, C], f32), ("keyC", [C, N], f32r),
        ("MzT", [C + 1, C + 1], f32r), ("wvT", [C, C], f32r),
        ("W1p", [C, H], f32r), ("w2", [H, C], f32r),
        ("c1t", [P, HT], f32), ("b2r", [1, C], f32r), ("onesr", [1, 512], f32r),
    ]:
        dI[nm] = nc.dram_tensor(nm, shp, dt, kind="ExternalInput").ap()
    outT = nc.dram_tensor("outT", [N, C], f32, kind="ExternalOutput").ap()
    x2d = nc.dram_tensor("x2d", [N, C], f32).ap()  # internal residual bounce

    with tile.TileContext(nc, pool_alloc_mode="queue") as tc, ExitStack() as ctx:
        # ---------- long-lived pools ----------
        psum = ctx.enter_context(tc.tile_pool(name="psum", bufs=6, space="PSUM"))
        _ctr = [0]

        def pt(shape, tag="ps", bufs=None):
            _ctr[0] += 1
            return psum.tile(shape, f32, tag=tag, name=f"pst{_ctr[0]}", bufs=bufs)

        const = ctx.enter_context(tc.tile_pool(name="const", bufs=1))
        stat_p = ctx.enter_context(tc.tile_pool(name="stat", bufs=8))
        load_p = ctx.enter_context(tc.tile_pool(name="load", bufs=4))
        work_p = ctx.enter_context(tc.tile_pool(name="work", bufs=8))

        ident = const.tile([P, P], f32)
        make_identity(nc, ident)
        ones_row = const.tile([1, 512], f32r)
        nc.sync.dma_start(ones_row, dI["onesr"])
        eps_col = const.tile([P, 1], f32)
        nc.vector.memset(eps_col, EPS)
        c1_sb = const.tile([P, HT], f32)
        nc.sync.dma_start(c1_sb, dI["c1t"])
        b2_sb = const.tile([1, C], f32r)
        nc.sync.dma_start(b2_sb, dI["b2r"])

        # ---------- helpers ----------
        def ln_of(s):
            st6 = stat_p.tile([P, 6], f32, tag="st6")
            nc.vector.bn_stats(st6, s)
            mv = stat_p.tile([P, 2], f32, tag="mv")
            nc.vector.bn_aggr(mv, st6)
            sd = stat_p.tile([P, 1], f32, tag="sd")
            nc.scalar.activation(sd, mv[:, 1:2], AF.Sqrt, bias=eps_col, scale=1.0)
            r = stat_p.tile([P, 1], f32, tag="r")
            nc.vector.reciprocal(r, sd)
            nmr = stat_p.tile([P, 1], f32, tag="nmr")
            nc.vector.tensor_scalar(out=nmr, in0=mv[:, 0:1], scalar1=r, scalar2=-1.0,
                                    op0=ALU.mult, op1=ALU.mult)
            xh = work_p.tile([P, C], f32, tag="xh")
            nc.scalar.activation(xh, s, AF.Identity, bias=nmr, scale=r)
            return xh

        def ln_xhat(src_a, src_b, nt_idx):
            a = load_p.tile([P, C], f32, tag="ld_a")
            nc.sync.dma_start(a, src_a[ts(nt_idx, P), :])
            b = load_p.tile([P, C], f32, tag="ld_b")
            nc.sync.dma_start(b, src_b[ts(nt_idx, P), :])
            s = work_p.tile([P, C], f32, tag="sum")
            nc.vector.tensor_add(s, a, b)
            return ln_of(s)

        def transpose_quad(xh4, ct, dst, dst_cols, evict="vector"):
            """Transpose the ct-th c-block of 4 T-tiles into dst[:, dst_cols]."""
            ps_t = pt([P, 512])
            for d, xh in enumerate(xh4):
                nc.tensor.transpose(ps_t[:, ts(d, P)], xh[:, ts(ct, P)], ident)
            if evict == "vector":
                nc.vector.tensor_copy(dst[:, dst_cols], ps_t)
            else:
                nc.scalar.copy(dst[:, dst_cols], ps_t)

        # LIFO pool discipline: kzp outlives the k-side scratch pools.
        kz_pool = tc.alloc_tile_pool(name="kzp", bufs=1)

        # ================= k side: xhat_k -> akrC -> kz =================
        mz_pool = tc.alloc_tile_pool(name="mzp", bufs=1)
        MzT_sb = []
        for kb in range(CT):
            t = mz_pool.tile([P, C + 1], f32r, name=f"MzT{kb}")
            nc.sync.dma_start(t, dI["MzT"][ts(kb, P), :])
            MzT_sb.append(t)
        MzT_last = mz_pool.tile([1, C + 1], f32r)
        nc.sync.dma_start(MzT_last, dI["MzT"][C:C + 1, :])

        akr_pool = tc.alloc_tile_pool(name="akrp", bufs=1)
        akrC = [akr_pool.tile([P, N], f32r, name=f"akr{ct}") for ct in range(CT)]
        for q4 in range(NT // 4):
            quad = [ln_xhat(dI["kT"], dI["keT"], q4 * 4 + d) for d in range(4)]
            for ct in range(CT):
                transpose_quad(quad, ct, akrC[ct], ts(q4, 512))

        kz_sb = [kz_pool.tile([P, N], f32r, name=f"kz{m}") for m in range(CT)]
        kz_last = kz_pool.tile([1, N], f32r)
        for m in range(CT + 1):
            small = m == CT
            m_sl = slice(C, C + 1) if small else ts(m, P)
            for ch in range(NCH):
                ps_k = pt([1 if small else P, 512],
                          tag="ps_s" if small else "ps", bufs=2 if small else None)
                for kb in range(CT):
                    nc.tensor.matmul(ps_k, MzT_sb[kb][:, m_sl],
                                     akrC[kb][:, ts(ch, 512)],
                                     start=(kb == 0), stop=False)
                nc.tensor.matmul(ps_k, MzT_last[:, m_sl], ones_row,
                                 start=False, stop=True)
                dst = kz_last if small else kz_sb[m]
                nc.scalar.copy(dst[:, ts(ch, 512)], ps_k)
        akr_pool.release()
        mz_pool.release()

        # ================= q side: xhat_q -> aqrC =================
        aqr_pool = tc.alloc_tile_pool(name="aqrp", bufs=1)
        aqrC = [aqr_pool.tile([P, N], f32r, name=f"aqr{ct}") for ct in range(CT)]
        for q4 in range(NT // 4):
            quad = [ln_xhat(dI["qT"], dI["qeT"], q4 * 4 + d) for d in range(4)]
            for ct in range(CT):
                transpose_quad(quad, ct, aqrC[ct], ts(q4, 512))

        # ================= vT conv (streamed keyC, chunk-interleaved) ========
        wv_pool = tc.alloc_tile_pool(name="wvp", bufs=1)
        key_pool = tc.alloc_tile_pool(name="keyp", bufs=6)
        vt_pool = tc.alloc_tile_pool(name="vtp", bufs=9)
        x2_pool = tc.alloc_tile_pool(name="x2p", bufs=4)
        attn_p = tc.alloc_tile_pool(name="attnp", bufs=3)
        halo_p = tc.alloc_tile_pool(name="halop", bufs=3)
        wvT_sb = []
        for kb in range(CT):
            t = wv_pool.tile([P, C], f32r, name=f"wvT{kb}")
            nc.sync.dma_start(t, dI["wvT"][ts(kb, P), :])
            wvT_sb.append(t)
        vT_tiles = []

        def emit_v_chunk(ch):
            kc = []
            for kb in range(CT):
                t = key_pool.tile([P, 512], f32r, tag="keyc")
                nc.sync.dma_start(t, dI["keyC"][ts(kb, P), ts(ch, 512)])
                kc.append(t)
            for sub in range(4):
                ps_v = pt([P, C])
                for kb in range(CT):
                    nc.tensor.matmul(ps_v, kc[kb][:, ts(sub, P)], wvT_sb[kb],
                                     start=(kb == 0), stop=(kb == CT - 1))
                vt = vt_pool.tile([P, C], f32r, tag="vt")
                nc.scalar.copy(vt, ps_v)
                vT_tiles.append(vt)

        emit_v_chunk(0)

        # ================= attention =================

        for nb in range(NT):
            if nb % 4 == 0 and nb // 4 + 1 < NCH:
                emit_v_chunk(nb // 4 + 1)
            fl = PADW if nb == 0 else 0
            fh = W - PADW if nb == NT - 1 else W
            wvd = fh - fl
            plo = nb * P - PADW + fl
            ps_g = pt([P, W])
            for kb in range(CT):
                nc.tensor.matmul(ps_g[:, fl:fh], aqrC[kb][:, ts(nb, P)],
                                 kz_sb[kb][:, plo:plo + wvd],
                                 start=(kb == 0), stop=False)
            nc.tensor.matmul(ps_g[:, fl:fh], ones_row[:, 0:P],
                             kz_last[:, plo:plo + wvd], start=False, stop=True)
            gs = attn_p.tile([P, W], f32, tag="gs")
            if fl > 0:
                nc.vector.memset(gs[:, 0:fl], 0.0)
            if fh < W:
                nc.vector.memset(gs[:, fh:W], 0.0)
            nc.scalar.copy(gs[:, fl:fh], ps_g[:, fl:fh])
            # band: keep where 0 <= f - p <= KH-1
            nc.gpsimd.affine_select(out=gs, in_=gs, pattern=[[1, W]], base=0,
                                    channel_multiplier=-1, compare_op=ALU.is_ge,
                                    fill=NEG)
            nc.gpsimd.affine_select(out=gs, in_=gs, pattern=[[-1, W]], base=KH - 1,
                                    channel_multiplier=1, compare_op=ALU.is_ge,
                                    fill=NEG)
            nmx = stat_p.tile([P, 1], f32, tag="nmx")
            nc.vector.reduce_max(out=nmx, in_=gs, axis=AX.X, negate=True)
            ge = attn_p.tile([P, W], f32, tag="ge")
            esum = stat_p.tile([P, 1], f32, tag="esum")
            nc.scalar.activation(ge, gs, AF.Exp, bias=nmx, scale=1.0,
                                 accum_out=esum)
            rsc = stat_p.tile([P, 1], f32, tag="rsc")
            nc.vector.reciprocal(rsc, esum)
            wn = attn_p.tile([P, W], f32, tag="wn")
            nc.vector.tensor_scalar(out=wn, in0=ge, scalar1=rsc, scalar2=scale,
                                    op0=ALU.mult, op1=ALU.mult)
            # transpose band pieces: A (cols 0:PADW -> prev-tile rows),
            # B (main), C (cols PADW+P:W -> next-tile rows)
            ps_w = pt([P, 3 * P])
            if nb > 0:
                nc.tensor.transpose(ps_w[0:PADW, 0:P], wn[:, 0:PADW], ident)
            nc.tensor.transpose(ps_w[:, P:2 * P], wn[:, PADW:PADW + P], ident)
            if nb < NT - 1:
                nc.tensor.transpose(ps_w[0:PADW, 2 * P:3 * P],
                                    wn[:, PADW + P:W], ident)
            wbB = attn_p.tile([P, P], f32r, tag="wbB")
            nc.vector.tensor_copy(wbB, ps_w[:, P:2 * P])
            if nb > 0:
                wbA = attn_p.tile([PADW, P], f32r, tag="wbA")
                nc.vector.tensor_copy(wbA, ps_w[0:PADW, 0:P])
            if nb < NT - 1:
                wbC = attn_p.tile([PADW, P], f32r, tag="wbC")
                nc.vector.tensor_copy(wbC, ps_w[0:PADW, 2 * P:3 * P])
            # weighting: attnT[p, c] = sum_{n'} Wb[n', p] * vT[n', c]
            ps_a = pt([P, C])
            first = True
            if nb > 0:
                halo = halo_p.tile([PADW, C], f32r, tag="halo")
                nc.sync.dma_start(halo, vT_tiles[nb - 1][P - PADW:P, :])
                nc.tensor.matmul(ps_a, wbA, halo, start=True, stop=False)
                first = False
            nc.tensor.matmul(ps_a, wbB, vT_tiles[nb],
                             start=first, stop=(nb == NT - 1))
            if nb < NT - 1:
                nc.tensor.matmul(ps_a, wbC, vT_tiles[nb + 1][0:PADW, :],
                                 start=False, stop=True)
            qtb = load_p.tile([P, C], f32, tag="qtb")
            nc.sync.dma_start(qtb, dI["qTb"][ts(nb, P), :])
            x2 = x2_pool.tile([P, C], f32, tag="x2")
            nc.vector.tensor_add(x2, ps_a, qtb)
            nc.sync.dma_start(x2d[ts(nb, P), :], x2)

        halo_p.release()
        attn_p.release()
        x2_pool.release()
        vt_pool.release()
        key_pool.release()
        wv_pool.release()
        aqr_pool.release()
        kz_pool.release()

        # ================= MLP phase =================
        mlpw = tc.alloc_tile_pool(name="mlpw", bufs=1)
        W1p_sb = []
        for kb in range(CT):
            t = mlpw.tile([P, H], f32r, name=f"W1p{kb}")
            nc.sync.dma_start(t, dI["W1p"][ts(kb, P), :])
            W1p_sb.append(t)
        w2_sb = []
        for kb in range(HT):
            t = mlpw.tile([P, C], f32r, name=f"w2_{kb}")
            nc.sync.dma_start(t, dI["w2"][ts(kb, P), :])
            w2_sb.append(t)

        x2r_pool = tc.alloc_tile_pool(name="x2rp", bufs=8)
        xh2c_pool = tc.alloc_tile_pool(name="xh2cp", bufs=8)
        hg_pool = tc.alloc_tile_pool(name="hgp", bufs=18)
        fin_pool = tc.alloc_tile_pool(name="finp", bufs=4)

        for ch in range(NCH):
            x2c = []
            for sub in range(4):
                nb = ch * 4 + sub
                t = x2r_pool.tile([P, C], f32, tag="x2r")
                nc.sync.dma_start(t, x2d[ts(nb, P), :])
                x2c.append(t)
            xh2 = [ln_of(t) for t in x2c]
            xh2c = []
            for ct in range(CT):
                dst = xh2c_pool.tile([P, 512], f32r, tag="xh2c")
                transpose_quad(xh2, ct, dst, slice(0, 512))
                xh2c.append(dst)
            # mm1 + gelu
            hg = []
            for m in range(HT):
                ps_h = pt([P, 512])
                for kb in range(CT):
                    nc.tensor.matmul(ps_h, W1p_sb[kb][:, ts(m, P)], xh2c[kb],
                                     start=(kb == 0), stop=(kb == CT - 1))
                hgt = hg_pool.tile([P, 512], f32r, tag="hg")
                nc.scalar.activation(hgt, ps_h, gelu_func,
                                     bias=c1_sb[:, m:m + 1], scale=1.0)
                hg.append(hgt)
            # mm2 (T-layout out) + b2 aug + residual
            for sub in range(4):
                nb = ch * 4 + sub
                ps_o = pt([P, C])
                for kb in range(HT):
                    nc.tensor.matmul(ps_o, hg[kb][:, ts(sub, P)], w2_sb[kb],
                                     start=(kb == 0), stop=False)
                nc.tensor.matmul(ps_o, ones_row[:, 0:P], b2_sb,
                                 start=False, stop=True)
                fin = fin_pool.tile([P, C], f32, tag="fin")
                nc.vector.tensor_add(fin, ps_o, x2c[sub])
                nc.sync.dma_start(outT[ts(nb, P), :], fin)

        fin_pool.release()
        hg_pool.release()
        xh2c_pool.release()
        x2r_pool.release()
        mlpw.release()

    return dI, outT


_CACHE = {}


def _get_compiled(N, KH, gelu_func=AF.Gelu):
    key = (N, KH, str(gelu_func))
    if key not in _CACHE:
        nc = bacc.Bacc("TRN2", target_bir_lowering=False, debug=False,
                       enable_asserts=False)
        build_block_kernel(nc, N, KH, gelu_func)
        nc.compile()
        _CACHE[key] = nc
    return _CACHE[key]


def host_prep(inputs, N, KH):
    """Fold weights and build the per-core input maps."""
    q = np.asarray(inputs["query"], np.float32)
    k = np.asarray(inputs["key"], np.float32)
    qe = np.asarray(inputs["query_embed"], np.float32)
    ke = np.asarray(inputs["key_embed"], np.float32)
    wq = np.asarray(inputs["wq"], np.float32)
    bq = np.asarray(inputs["bq"], np.float32)
    wk = np.asarray(inputs["wk"], np.float32)
    bk = np.asarray(inputs["bk"], np.float32)
    wv = np.asarray(inputs["wv"], np.float32)
    bv = np.asarray(inputs["bv"], np.float32)
    g = np.asarray(inputs["g_norm"], np.float32)
    b = np.asarray(inputs["b_norm"], np.float32)
    g2 = np.asarray(inputs["g_norm2"], np.float32)
    b2n = np.asarray(inputs["b_norm2"], np.float32)
    w1 = np.asarray(inputs["w1"], np.float32)
    b1 = np.asarray(inputs["b1"], np.float32)
    w2 = np.asarray(inputs["w2"], np.float32)
    b2 = np.asarray(inputs["b2"], np.float32)

    Bsz = q.shape[0]
    scale = C ** -0.5

    Aq = np.concatenate([wq * g[None, :], (wq @ b + bq)[:, None]], axis=1)
    Ak = np.concatenate([wk * g[None, :], (wk @ b + bk)[:, None]], axis=1)
    MzT = np.ascontiguousarray(Ak.T @ Aq)  # lhsT for kz = Mz @ z_k, Mz = Aq^T Ak

    W1p = np.ascontiguousarray(w1 * g2[:, None])
    c1 = b2n @ w1 + b1
    c1t = np.ascontiguousarray(c1.reshape(HT, P).T)
    shared = {
        "MzT": MzT,
        "wvT": np.ascontiguousarray(wv.T),
        "W1p": W1p,
        "w2": np.ascontiguousarray(w2),
        "c1t": c1t,
        "b2r": np.ascontiguousarray(b2[None, :]),
        "onesr": np.ones((1, 512), np.float32),
    }
    in_maps = []
    for i in range(Bsz):
        m = dict(shared)
        m["qT"] = np.ascontiguousarray(q[i].T)
        m["qeT"] = np.ascontiguousarray(qe[i].T)
        m["kT"] = np.ascontiguousarray(k[i].T)
        m["keT"] = np.ascontiguousarray(ke[i].T)
        m["qTb"] = np.ascontiguousarray(q[i].T + scale * bv[None, :])
        m["keyC"] = np.ascontiguousarray(k[i])
        in_maps.append(m)
    return in_maps


def kernel(**inputs):
    q = np.asarray(inputs["query"])
    Bsz, Cin, N = q.shape
    assert Cin == C, f"built for C={C}"
    KH = int(inputs["kH"])
    nc = _get_compiled(N, KH)
    in_maps = host_prep(inputs, N, KH)
    core_ids = list(range(len(in_maps)))
    res = run_bass_kernel_spmd(nc, in_maps, core_ids)
    out = np.stack([np.ascontiguousarray(r["outT"].T) for r in res.results], axis=0)
    return out.astype(np.float32)


if __name__ == "__main__":
    _get_compiled(2048, 9)
    print("built + compiled OK")
